# revision 30
# baseline (speedup 1.0000x reference)
"""MoE routing gather kernel for Trainium2 (8 NeuronCores, data-parallel).

Math (per token t with K=8 slots, E=64 experts, D=512):
    path[t] = sum_k w[t,k] * V[idx[t,k]] / sum_k w[t,k]
    efficiency = mean_t ||path[t]||_2

Device algorithm per core (B=8192 tokens), "expert-slab" formulation:
  - Partition layout p = 32*k' + e32 packs 4 k-slots x 32 experts; with two
    k-groups (kg) and two expert slabs (s) there are 4 weighted one-hot
    planes P[kg][s][p, t] = w[t, 4kg+k'] * (idx[t, 4kg+k'] == 32s+e32).
    Each is ONE fused scalar_tensor_tensor (is_equal -> mult) over
    host-replicated idx/w planes (x32 replication done on host; planes are
    DMA'd straight from DRAM, no on-chip broadcast). k-groups are added in
    place (DVE) -> Ps[s].
  - out_chunk[128t, 512] = sum_s Ps[s]^T @ Vstack[s] (V rows tiled x4; the
    PE contraction over 128 partitions sums the 4 k-slots, PSUM accumulates
    the 2 slabs): 2 matmuls of N=512 per 128-token chunk.
  - Row norms via Cholesky Gram trick: with L = chol(V V^T) (host-computed
    from the replicated table), ||unnorm[t]||^2 = ||S_row L||^2; one extra
    N=64 matmul per (chunk, slab) (rhs = Lhat[s]) gives Y = S L, then
    squares+reduce on DVE.
  - Normalization 1/sum_k w folds into the ScalarE PSUM evacuation
    (activation Copy with per-partition scale); output stored bf16 in
    chunk-major layout [p, (c, d)] (contiguous 8KB store runs) and
    un-permuted/cast to f32 on host (rel err ~2.7e-3, tolerance 2e-2).
  - Each core writes its path shard and a [128, 1] partial sum of row norms
    r*sqrt(q); the host sums partials / 65536 for the efficiency scalar.
"""

import sys

sys.path.insert(0, "/opt/trn_rl_repo")

import numpy as np
import ml_dtypes

B_TOTAL = 65536
N_CORES = 8
B = B_TOTAL // N_CORES  # 8192 tokens per core
K = 8
E = 64
D = 512
SB = 8       # chunks per sub-batch (store + Y-psum granularity)

CFG = dict(
    n_groups=8,          # token groups per core (plane-load granularity)
    out_bf16=True,       # write path output in bf16 (host casts to f32)
    trace=False,         # capture neuron profile (exec_time_ns)
)

_COMPILED = {}
LAST_RESULT = {}


def _install_ntff_shim():
    """Make run_bass_kernel_spmd(trace=True) work under axon: register the
    antenv.axon_hooks module (absent in this image) with the ctypes-based
    NTFF profile hook, and keep artifacts local."""
    import types

    if "antenv.axon_hooks" not in sys.modules:
        sys.path.insert(0, "/root/.axon_site")
        from trn_agent_boot.trn_boot import _ntff_profile_via_ctypes

        hook = _ntff_profile_via_ctypes("/opt/axon/libaxon_pjrt.so")
        mod = types.ModuleType("antenv.axon_hooks")
        store = [hook]
        mod.set_axon_ntff_profile_hook = lambda h: store.__setitem__(0, h)
        mod.get_axon_ntff_profile_hook = lambda: store[0]
        sys.modules["antenv.axon_hooks"] = mod
        import antenv

        antenv.axon_hooks = mod
    import concourse.bass_utils as bu

    bu.upload_artifacts = lambda d: d


def _build(cfg):
    import concourse.bass as bass
    import concourse.mybir as mybir
    import concourse.tile as tile
    from concourse import bacc

    dt = mybir.dt
    f32 = dt.float32
    bf16 = dt.bfloat16
    AX = mybir.AxisListType
    OP = mybir.AluOpType

    NG = cfg["n_groups"]
    TB = B // NG          # tokens per group
    CH = TB // 128        # chunks (of 128 tokens) per group
    NSB = CH // SB
    SBT = SB * 128        # tokens per sub-batch
    out_dt = bf16 if cfg["out_bf16"] else f32

    nc = bacc.Bacc("TRN2", target_bir_lowering=False, debug=False,
                   num_devices=N_CORES)

    u8 = dt.uint8
    ips, wps = [], []
    for kg in range(2):
        ips.append(nc.dram_tensor(f"ip{kg}", [128, B], u8,
                                  kind="ExternalInput"))
        wps.append(nc.dram_tensor(f"wp{kg}", [128, B], u8,
                                  kind="ExternalInput"))
    wtok_d = nc.dram_tensor("wtok", [128, (B // 128) * K], bf16,
                            kind="ExternalInput")
    ecol2_d = nc.dram_tensor("ecol2", [128, 2], f32, kind="ExternalInput")
    lhat_d = [nc.dram_tensor(f"lhat{s}", [128, E], bf16,
                             kind="ExternalInput") for s in range(2)]
    vst_d = [nc.dram_tensor(f"vst{s}", [128, D], bf16, kind="ExternalInput")
             for s in range(2)]
    # chunk-major: [p, (c, d)] with token t = c*128 + p; host un-permutes.
    path_d = nc.dram_tensor("path_out", [128, (B // 128) * D], out_dt,
                            kind="ExternalOutput")
    eff_d = nc.dram_tensor("eff_out", [128, 1], f32, kind="ExternalOutput")

    with tile.TileContext(nc) as tc:
        import contextlib
        with contextlib.ExitStack() as ctx:
            const_p = ctx.enter_context(tc.tile_pool(name="const", bufs=1))
            bc_p = ctx.enter_context(tc.tile_pool(name="bc", bufs=3))
            pp_p = ctx.enter_context(tc.tile_pool(name="pp", bufs=3))
            stage_p = ctx.enter_context(tc.tile_pool(name="stage", bufs=3))
            small_p = ctx.enter_context(tc.tile_pool(name="small", bufs=2))
            ps_out = ctx.enter_context(
                tc.tile_pool(name="ps_out", bufs=4, space="PSUM"))
            ps_y = ctx.enter_context(
                tc.tile_pool(name="ps_y", bufs=2, space="PSUM"))

            # ---------------- setup ----------------
            # constants ride the scalar HWDGE ring (idle early); the sync
            # ring starts streaming the first plane slices immediately
            ecol2 = const_p.tile([128, 2], f32)
            nc.scalar.dma_start(ecol2[:], ecol2_d.ap())
            wtok = const_p.tile([128, (B // 128) * K], bf16)
            nc.scalar.dma_start(wtok[:, 0:(B // 128) * K // 2],
                                wtok_d.ap()[:, 0:(B // 128) * K // 2])
            nc.scalar.dma_start(wtok[:, (B // 128) * K // 2:],
                                wtok_d.ap()[:, (B // 128) * K // 2:])
            vst, lhat = [], []
            for s in range(2):
                t = const_p.tile([128, D], bf16, tag=f"vst{s}")
                nc.scalar.dma_start(t[:], vst_d[s].ap())
                vst.append(t)
                t2 = const_p.tile([128, E], bf16, tag=f"lhat{s}")
                nc.scalar.dma_start(t2[:], lhat_d[s].ap())
                lhat.append(t2)

            tw = const_p.tile([128, B // 128], f32)
            r = const_p.tile([128, B // 128], f32)
            qraw = const_p.tile([128, B // 128], f32)
            # preload the sqrt activation table before the hot loop
            sq_warm = const_p.tile([128, 1], f32)
            nc.scalar.sqrt(sq_warm[:], ecol2[:, 0:1])

            # ---------------- main loop ----------------
            for g in range(NG):
                t0 = g * TB
                ip_sb, wp_sb = [], []
                for kg in range(2):
                    ti = bc_p.tile([128, TB], u8, tag=f"ip{kg}")
                    tww = bc_p.tile([128, TB], u8, tag=f"wp{kg}")
                    for sbi in range(NSB):
                        sl = slice(sbi * SBT, (sbi + 1) * SBT)
                        gsl = slice(t0 + sbi * SBT, t0 + (sbi + 1) * SBT)
                        nc.sync.dma_start(ti[:, sl], ips[kg].ap()[:, gsl])
                        nc.sync.dma_start(tww[:, sl], wps[kg].ap()[:, gsl])
                    ip_sb.append(ti)
                    wp_sb.append(tww)

                # per-group normalization factors (keeps the DVE queue
                # from serializing behind the whole wtok load)
                gc = slice(g * CH, (g + 1) * CH)
                nc.vector.tensor_reduce(
                    tw[:, gc],
                    wtok[:, g * CH * K:(g + 1) * CH * K]
                    .rearrange("p (c k) -> p c k", k=K),
                    axis=AX.X, op=OP.add)
                nc.vector.reciprocal(r[:, gc], tw[:, gc])

                pk = [[pp_p.tile([128, TB], bf16, tag=f"pj{kg}{s}",
                                 name=f"pj{g}_{kg}_{s}")
                       for kg in range(2)] for s in range(2)]
                # sub-batch-outer emission so the first chunk's matmuls
                # unblock as early as possible
                for sbi in range(NSB):
                    sl = slice(sbi * SBT, (sbi + 1) * SBT)
                    for s in range(2):
                        for kg in range(2):
                            nc.vector.scalar_tensor_tensor(
                                pk[s][kg][:, sl], ip_sb[kg][:, sl],
                                ecol2[:, s:s + 1], wp_sb[kg][:, sl],
                                op0=OP.is_equal, op1=OP.mult)
                        # in-place: pk[s][0] += pk[s][1] = the slab plane
                        nc.vector.tensor_tensor(
                            pk[s][0][:, sl], pk[s][0][:, sl],
                            pk[s][1][:, sl], op=OP.add)
                ps = [pk[0][0], pk[1][0]]

                for sbi in range(NSB):
                    sc0 = sbi * SB
                    c0 = g * CH + sc0
                    stage = stage_p.tile([128, SB * D], out_dt, tag="stage")
                    yq = ps_y.tile([128, SB * E], f32, tag="yq")
                    for cc in range(SB):
                        off = (sc0 + cc) * 128
                        po = ps_out.tile([128, D], f32, tag="po")
                        # lhsT (Ps[s] chunk) shared by the main and Y
                        # matmuls -> emitted adjacently
                        for s in range(2):
                            nc.tensor.matmul(
                                po[:], ps[s][:, off:off + 128], vst[s][:],
                                start=(s == 0), stop=(s == 1))
                            nc.tensor.matmul(
                                yq[:, cc * E:(cc + 1) * E],
                                ps[s][:, off:off + 128], lhat[s][:],
                                start=(s == 0), stop=(s == 1))
                        nc.scalar.mul(
                            stage[:, cc * D:(cc + 1) * D], po[:],
                            r[:, c0 + cc:c0 + cc + 1])
                    ysq = small_p.tile([128, SB * E], bf16, tag="ysq")
                    nc.scalar.square(ysq[:], yq[:])
                    nc.vector.tensor_reduce(
                        qraw[:, c0:c0 + SB],
                        ysq[:].rearrange("p (c e) -> p c e", e=E),
                        axis=AX.X, op=OP.add)
                    nc.scalar.dma_start(
                        path_d.ap()[:, c0 * D:(c0 + SB) * D], stage[:])

            # ---------------- efficiency partials ----------------
            sqn = const_p.tile([128, B // 128], f32)
            nc.scalar.sqrt(sqn[:], qraw[:])
            norms = const_p.tile([128, B // 128], f32)
            nc.vector.tensor_tensor(norms[:], sqn[:], r[:], op=OP.mult)
            effp = const_p.tile([128, 1], f32)
            nc.vector.tensor_reduce(effp[:], norms[:], axis=AX.X, op=OP.add)
            nc.sync.dma_start(eff_d.ap(), effp[:])

    nc.compile()
    return nc


def _get_nc(cfg_key):
    if cfg_key not in _COMPILED:
        _COMPILED[cfg_key] = _build(dict(cfg_key))
    return _COMPILED[cfg_key]


def kernel(expert_indices, expert_weights, vertices):
    from concourse.bass_utils import run_bass_kernel_spmd

    cfg = dict(CFG)
    cfg_key = tuple(sorted(cfg.items()))
    nc = _get_nc(cfg_key)

    bf = ml_dtypes.bfloat16
    idx = np.asarray(expert_indices)
    w = np.asarray(expert_weights, dtype=np.float32)
    v = np.asarray(vertices, dtype=np.float32)

    ecol2 = np.stack(
        [32.0 * s + (np.arange(128, dtype=np.float32) % 32)
         for s in range(2)], axis=1)
    vst = [np.tile(v[32 * s:32 * (s + 1)], (4, 1)).astype(bf)
           for s in range(2)]
    # Cholesky factor of the vertex Gram matrix (auxiliary constant for the
    # row-norm path: ||S V||^2 = ||S L||^2 with G = V V^T = L L^T)
    g = (v.astype(np.float64) @ v.astype(np.float64).T)
    lch = np.linalg.cholesky(g + 1e-6 * np.eye(E))
    lhat = [np.ascontiguousarray(
        lch[(32 * s + np.arange(128) % 32)]).astype(bf) for s in range(2)]

    in_maps = []
    for c in range(N_CORES):
        sl = slice(c * B, (c + 1) * B)
        idx_t = np.ascontiguousarray(idx[sl].T).astype(np.uint8)  # [K, B]
        # weights quantized to u8 (x255); u8 ints are exact in bf16, so the
        # x255 scale cancels exactly through r = 1/sum(wq)
        wq = np.rint(w[sl] * 255.0).astype(np.uint8)              # [B, K]
        wq_t = np.ascontiguousarray(wq.T)                         # [K, B]
        m = {
            "wtok": np.ascontiguousarray(
                wq.astype(np.float32).reshape(B // 128, 128, K)
                .transpose(1, 0, 2)
                .reshape(128, (B // 128) * K)).astype(bf),
            "ecol2": ecol2,
            "lhat0": lhat[0], "lhat1": lhat[1],
            "vst0": vst[0], "vst1": vst[1],
        }
        for kg in range(2):
            m[f"ip{kg}"] = np.ascontiguousarray(
                np.repeat(idx_t[4 * kg:4 * kg + 4], 32, axis=0))
            m[f"wp{kg}"] = np.ascontiguousarray(
                np.repeat(wq_t[4 * kg:4 * kg + 4], 32, axis=0))
        in_maps.append(m)

    tmpdir = None
    if cfg["trace"]:
        import tempfile

        _install_ntff_shim()
        tmpdir = tempfile.mkdtemp(prefix="moe_trace_")
    res = run_bass_kernel_spmd(
        nc, in_maps, core_ids=list(range(N_CORES)), trace=cfg["trace"],
        tmpdir=tmpdir)
    LAST_RESULT["exec_time_ns"] = res.exec_time_ns
    LAST_RESULT["mean_exec_time_ns"] = res.mean_exec_time_ns
    LAST_RESULT["trace_dir"] = tmpdir

    path = np.concatenate(
        [np.asarray(res.results[c]["path_out"], dtype=np.float32)
         .reshape(128, B // 128, D).transpose(1, 0, 2).reshape(B, D)
         for c in range(N_CORES)], axis=0)
    eff = np.float32(
        sum(float(np.asarray(res.results[c]["eff_out"], dtype=np.float64).sum())
            for c in range(N_CORES)) / B_TOTAL)
    return path, eff


# revision 31
# speedup vs baseline: 1.0351x; 1.0351x over previous
"""MoE routing gather kernel for Trainium2 (8 NeuronCores, data-parallel).

Math (per token t with K=8 slots, E=64 experts, D=512):
    path[t] = sum_k w[t,k] * V[idx[t,k]] / sum_k w[t,k]
    efficiency = mean_t ||path[t]||_2

Device algorithm per core (B=8192 tokens), "expert-slab" formulation:
  - Partition layout p = 32*k' + e32 packs 4 k-slots x 32 experts; with two
    k-groups (kg) and two expert slabs (s) there are 4 weighted one-hot
    planes P[kg][s][p, t] = w[t, 4kg+k'] * (idx[t, 4kg+k'] == 32s+e32).
    Each is ONE fused scalar_tensor_tensor (is_equal -> mult) over
    host-replicated idx/w planes (x32 replication done on host; planes are
    DMA'd straight from DRAM, no on-chip broadcast). k-groups are added in
    place (DVE) -> Ps[s].
  - out_chunk[128t, 512] = sum_s Ps[s]^T @ Vstack[s] (V rows tiled x4; the
    PE contraction over 128 partitions sums the 4 k-slots, PSUM accumulates
    the 2 slabs): 2 matmuls of N=512 per 128-token chunk.
  - Row norms via Cholesky Gram trick: with L = chol(V V^T) (host-computed
    from the replicated table), ||unnorm[t]||^2 = ||S_row L||^2; one extra
    N=64 matmul per (chunk, slab) (rhs = Lhat[s]) gives Y = S L, then
    squares+reduce on DVE.
  - Normalization 1/sum_k w folds into the ScalarE PSUM evacuation
    (activation Copy with per-partition scale); output stored bf16 in
    chunk-major layout [p, (c, d)] (contiguous 8KB store runs) and
    un-permuted/cast to f32 on host (rel err ~2.7e-3, tolerance 2e-2).
  - Each core writes its path shard and a [128, 1] partial sum of row norms
    r*sqrt(q); the host sums partials / 65536 for the efficiency scalar.
"""

import sys

sys.path.insert(0, "/opt/trn_rl_repo")

import numpy as np
import ml_dtypes

B_TOTAL = 65536
N_CORES = 8
B = B_TOTAL // N_CORES  # 8192 tokens per core
K = 8
E = 64
D = 512
SB = 8       # chunks per sub-batch (store + Y-psum granularity)

CFG = dict(
    n_groups=8,          # token groups per core (plane-load granularity)
    out_bf16=True,       # write path output in bf16 (host casts to f32)
    trace=False,         # capture neuron profile (exec_time_ns)
)

_COMPILED = {}
LAST_RESULT = {}


def _install_ntff_shim():
    """Make run_bass_kernel_spmd(trace=True) work under axon: register the
    antenv.axon_hooks module (absent in this image) with the ctypes-based
    NTFF profile hook, and keep artifacts local."""
    import types

    if "antenv.axon_hooks" not in sys.modules:
        sys.path.insert(0, "/root/.axon_site")
        from trn_agent_boot.trn_boot import _ntff_profile_via_ctypes

        hook = _ntff_profile_via_ctypes("/opt/axon/libaxon_pjrt.so")
        mod = types.ModuleType("antenv.axon_hooks")
        store = [hook]
        mod.set_axon_ntff_profile_hook = lambda h: store.__setitem__(0, h)
        mod.get_axon_ntff_profile_hook = lambda: store[0]
        sys.modules["antenv.axon_hooks"] = mod
        import antenv

        antenv.axon_hooks = mod
    import concourse.bass_utils as bu

    bu.upload_artifacts = lambda d: d


def _build(cfg):
    import concourse.bass as bass
    import concourse.mybir as mybir
    import concourse.tile as tile
    from concourse import bacc

    dt = mybir.dt
    f32 = dt.float32
    bf16 = dt.bfloat16
    AX = mybir.AxisListType
    OP = mybir.AluOpType

    NG = cfg["n_groups"]
    TB = B // NG          # tokens per group
    CH = TB // 128        # chunks (of 128 tokens) per group
    NSB = CH // SB
    SBT = SB * 128        # tokens per sub-batch
    out_dt = bf16 if cfg["out_bf16"] else f32

    nc = bacc.Bacc("TRN2", target_bir_lowering=False, debug=False,
                   num_devices=N_CORES)

    u8 = dt.uint8
    ips, wps = [], []
    for kg in range(2):
        ips.append(nc.dram_tensor(f"ip{kg}", [128, B], u8,
                                  kind="ExternalInput"))
        wps.append(nc.dram_tensor(f"wp{kg}", [128, B], u8,
                                  kind="ExternalInput"))
    wtok_d = nc.dram_tensor("wtok", [128, (B // 128) * K], bf16,
                            kind="ExternalInput")
    ecol2_d = nc.dram_tensor("ecol2", [128, 2], f32, kind="ExternalInput")
    lhat_d = [nc.dram_tensor(f"lhat{s}", [128, E], bf16,
                             kind="ExternalInput") for s in range(2)]
    vst_d = [nc.dram_tensor(f"vst{s}", [128, D], bf16, kind="ExternalInput")
             for s in range(2)]
    # chunk-major: [p, (c, d)] with token t = c*128 + p; host un-permutes.
    path_d = nc.dram_tensor("path_out", [128, (B // 128) * D], out_dt,
                            kind="ExternalOutput")
    eff_d = nc.dram_tensor("eff_out", [128, 1], f32, kind="ExternalOutput")

    with tile.TileContext(nc) as tc:
        import contextlib
        with contextlib.ExitStack() as ctx:
            const_p = ctx.enter_context(tc.tile_pool(name="const", bufs=1))
            bc_p = ctx.enter_context(tc.tile_pool(name="bc", bufs=3))
            pp_p = ctx.enter_context(tc.tile_pool(name="pp", bufs=3))
            stage_p = ctx.enter_context(tc.tile_pool(name="stage", bufs=4))
            small_p = ctx.enter_context(tc.tile_pool(name="small", bufs=3))
            ps_out = ctx.enter_context(
                tc.tile_pool(name="ps_out", bufs=5, space="PSUM"))
            ps_y = ctx.enter_context(
                tc.tile_pool(name="ps_y", bufs=2, space="PSUM"))

            # ---------------- setup ----------------
            # constants ride the scalar HWDGE ring (idle early); the sync
            # ring starts streaming the first plane slices immediately
            ecol2 = const_p.tile([128, 2], f32)
            nc.scalar.dma_start(ecol2[:], ecol2_d.ap())
            wtok = const_p.tile([128, (B // 128) * K], bf16)
            nc.scalar.dma_start(wtok[:, 0:(B // 128) * K // 2],
                                wtok_d.ap()[:, 0:(B // 128) * K // 2])
            nc.scalar.dma_start(wtok[:, (B // 128) * K // 2:],
                                wtok_d.ap()[:, (B // 128) * K // 2:])
            vst, lhat = [], []
            for s in range(2):
                t = const_p.tile([128, D], bf16, tag=f"vst{s}")
                nc.scalar.dma_start(t[:], vst_d[s].ap())
                vst.append(t)
                t2 = const_p.tile([128, E], bf16, tag=f"lhat{s}")
                nc.scalar.dma_start(t2[:], lhat_d[s].ap())
                lhat.append(t2)

            tw = const_p.tile([128, B // 128], f32)
            r = const_p.tile([128, B // 128], f32)
            qraw = const_p.tile([128, B // 128], f32)
            # preload the sqrt activation table before the hot loop
            sq_warm = const_p.tile([128, 1], f32)
            nc.scalar.sqrt(sq_warm[:], ecol2[:, 0:1])

            # ---------------- main loop ----------------
            for g in range(NG):
                t0 = g * TB
                ip_sb, wp_sb = [], []
                for kg in range(2):
                    ti = bc_p.tile([128, TB], u8, tag=f"ip{kg}")
                    tww = bc_p.tile([128, TB], u8, tag=f"wp{kg}")
                    for sbi in range(NSB):
                        sl = slice(sbi * SBT, (sbi + 1) * SBT)
                        gsl = slice(t0 + sbi * SBT, t0 + (sbi + 1) * SBT)
                        nc.sync.dma_start(ti[:, sl], ips[kg].ap()[:, gsl])
                        nc.sync.dma_start(tww[:, sl], wps[kg].ap()[:, gsl])
                    ip_sb.append(ti)
                    wp_sb.append(tww)

                # per-group normalization factors (keeps the DVE queue
                # from serializing behind the whole wtok load)
                gc = slice(g * CH, (g + 1) * CH)
                nc.vector.tensor_reduce(
                    tw[:, gc],
                    wtok[:, g * CH * K:(g + 1) * CH * K]
                    .rearrange("p (c k) -> p c k", k=K),
                    axis=AX.X, op=OP.add)
                nc.vector.reciprocal(r[:, gc], tw[:, gc])

                pk = [[pp_p.tile([128, TB], bf16, tag=f"pj{kg}{s}",
                                 name=f"pj{g}_{kg}_{s}")
                       for kg in range(2)] for s in range(2)]
                # sub-batch-outer emission so the first chunk's matmuls
                # unblock as early as possible
                for sbi in range(NSB):
                    sl = slice(sbi * SBT, (sbi + 1) * SBT)
                    for s in range(2):
                        for kg in range(2):
                            nc.vector.scalar_tensor_tensor(
                                pk[s][kg][:, sl], ip_sb[kg][:, sl],
                                ecol2[:, s:s + 1], wp_sb[kg][:, sl],
                                op0=OP.is_equal, op1=OP.mult)
                        # in-place: pk[s][0] += pk[s][1] = the slab plane
                        nc.vector.tensor_tensor(
                            pk[s][0][:, sl], pk[s][0][:, sl],
                            pk[s][1][:, sl], op=OP.add)
                ps = [pk[0][0], pk[1][0]]

                for sbi in range(NSB):
                    sc0 = sbi * SB
                    c0 = g * CH + sc0
                    stage = stage_p.tile([128, SB * D], out_dt, tag="stage")
                    yq = ps_y.tile([128, SB * E], f32, tag="yq")
                    for cc in range(SB):
                        off = (sc0 + cc) * 128
                        po = ps_out.tile([128, D], f32, tag="po")
                        # lhsT (Ps[s] chunk) shared by the main and Y
                        # matmuls -> emitted adjacently
                        for s in range(2):
                            nc.tensor.matmul(
                                po[:], ps[s][:, off:off + 128], vst[s][:],
                                start=(s == 0), stop=(s == 1))
                            nc.tensor.matmul(
                                yq[:, cc * E:(cc + 1) * E],
                                ps[s][:, off:off + 128], lhat[s][:],
                                start=(s == 0), stop=(s == 1))
                        nc.scalar.mul(
                            stage[:, cc * D:(cc + 1) * D], po[:],
                            r[:, c0 + cc:c0 + cc + 1])
                    ysq = small_p.tile([128, SB * E], bf16, tag="ysq")
                    nc.scalar.square(ysq[:], yq[:])
                    nc.vector.tensor_reduce(
                        qraw[:, c0:c0 + SB],
                        ysq[:].rearrange("p (c e) -> p c e", e=E),
                        axis=AX.X, op=OP.add)
                    nc.scalar.dma_start(
                        path_d.ap()[:, c0 * D:(c0 + SB) * D], stage[:])

            # ---------------- efficiency partials ----------------
            sqn = const_p.tile([128, B // 128], f32)
            nc.scalar.sqrt(sqn[:], qraw[:])
            norms = const_p.tile([128, B // 128], f32)
            nc.vector.tensor_tensor(norms[:], sqn[:], r[:], op=OP.mult)
            effp = const_p.tile([128, 1], f32)
            nc.vector.tensor_reduce(effp[:], norms[:], axis=AX.X, op=OP.add)
            nc.sync.dma_start(eff_d.ap(), effp[:])

    nc.compile()
    return nc


def _get_nc(cfg_key):
    if cfg_key not in _COMPILED:
        _COMPILED[cfg_key] = _build(dict(cfg_key))
    return _COMPILED[cfg_key]


def kernel(expert_indices, expert_weights, vertices):
    from concourse.bass_utils import run_bass_kernel_spmd

    cfg = dict(CFG)
    cfg_key = tuple(sorted(cfg.items()))
    nc = _get_nc(cfg_key)

    bf = ml_dtypes.bfloat16
    idx = np.asarray(expert_indices)
    w = np.asarray(expert_weights, dtype=np.float32)
    v = np.asarray(vertices, dtype=np.float32)

    ecol2 = np.stack(
        [32.0 * s + (np.arange(128, dtype=np.float32) % 32)
         for s in range(2)], axis=1)
    vst = [np.tile(v[32 * s:32 * (s + 1)], (4, 1)).astype(bf)
           for s in range(2)]
    # Cholesky factor of the vertex Gram matrix (auxiliary constant for the
    # row-norm path: ||S V||^2 = ||S L||^2 with G = V V^T = L L^T)
    g = (v.astype(np.float64) @ v.astype(np.float64).T)
    lch = np.linalg.cholesky(g + 1e-6 * np.eye(E))
    lhat = [np.ascontiguousarray(
        lch[(32 * s + np.arange(128) % 32)]).astype(bf) for s in range(2)]

    in_maps = []
    for c in range(N_CORES):
        sl = slice(c * B, (c + 1) * B)
        idx_t = np.ascontiguousarray(idx[sl].T).astype(np.uint8)  # [K, B]
        # weights quantized to u8 (x255); u8 ints are exact in bf16, so the
        # x255 scale cancels exactly through r = 1/sum(wq)
        wq = np.rint(w[sl] * 255.0).astype(np.uint8)              # [B, K]
        wq_t = np.ascontiguousarray(wq.T)                         # [K, B]
        m = {
            "wtok": np.ascontiguousarray(
                wq.astype(np.float32).reshape(B // 128, 128, K)
                .transpose(1, 0, 2)
                .reshape(128, (B // 128) * K)).astype(bf),
            "ecol2": ecol2,
            "lhat0": lhat[0], "lhat1": lhat[1],
            "vst0": vst[0], "vst1": vst[1],
        }
        for kg in range(2):
            m[f"ip{kg}"] = np.ascontiguousarray(
                np.repeat(idx_t[4 * kg:4 * kg + 4], 32, axis=0))
            m[f"wp{kg}"] = np.ascontiguousarray(
                np.repeat(wq_t[4 * kg:4 * kg + 4], 32, axis=0))
        in_maps.append(m)

    tmpdir = None
    if cfg["trace"]:
        import tempfile

        _install_ntff_shim()
        tmpdir = tempfile.mkdtemp(prefix="moe_trace_")
    res = run_bass_kernel_spmd(
        nc, in_maps, core_ids=list(range(N_CORES)), trace=cfg["trace"],
        tmpdir=tmpdir)
    LAST_RESULT["exec_time_ns"] = res.exec_time_ns
    LAST_RESULT["mean_exec_time_ns"] = res.mean_exec_time_ns
    LAST_RESULT["trace_dir"] = tmpdir

    path = np.concatenate(
        [np.asarray(res.results[c]["path_out"], dtype=np.float32)
         .reshape(128, B // 128, D).transpose(1, 0, 2).reshape(B, D)
         for c in range(N_CORES)], axis=0)
    eff = np.float32(
        sum(float(np.asarray(res.results[c]["eff_out"], dtype=np.float64).sum())
            for c in range(N_CORES)) / B_TOTAL)
    return path, eff
